# revision 14
# baseline (speedup 1.0000x reference)
"""TopK autoencoder (B=4096, D=1024, F=32768, K=64) on 8 Trainium2 NeuronCores.

Strategy: data-parallel over batch (512 rows/core). Per core, per 128-row tile:
  A) encoder matmul (PE): fp16 main pass (xh.Wh) + fp8e4m3 DoubleRow residual
     pass computing 2^15*(xl.Wh + xh.Wl) via pre-scaled operands; combined on
     ACT/DVE (pre-act abs err ~7e-6) and spilled raw to DRAM in f32; fused
     per-group (32 elems) running max on DVE.
  B) top-K: group maxima clamped at 0 and packed as
     (fp16-value-bits << 16 | group-id) so max8/match_replace rounds are
     tie-free; top-80 groups gathered from the spilled pre-activations by
     per-column indirect DMAs (multi-offset indirect DMA crashes the HW
     runtime); candidates packed the same way with element tags; K-th
     largest of the f32 candidates = threshold; masked packed rounds
     extract the top-64 (value, index) pairs.
  C) decode: gather the selected W_dec rows (fp16) by index per-column and
     accumulate w_k * row_k on the PE via diagonal-matrix matmuls into PSUM.
b_dec is handled exactly on the host (x - b_dec, + b_dec at the end); a
nonzero b_enc is folded in as an extra contraction tile (zero here).
"""
import sys
sys.path.insert(0, '/opt/trn_rl_repo')
import numpy as np
import concourse.bass as bass
import concourse.mybir as mybir
from concourse import bacc
from concourse.bass import ts, ds
from concourse.tile import TileContext
from concourse.masks import make_identity
from concourse.bass_utils import run_bass_kernel_spmd

f32 = mybir.dt.float32
f16 = mybir.dt.float16
f8 = mybir.dt.float8e4
u16 = mybir.dt.uint16
u32 = mybir.dt.uint32
i32 = mybir.dt.int32
Alu = mybir.AluOpType
Act = mybir.ActivationFunctionType

B, D, F, K = 4096, 1024, 32768, 64
N_CORES = 8
GE = 32     # group size
NP = 80     # candidate groups per row (64 needed + tie margin)
PB = 8      # decode gather block (rows per indirect DMA)

_CACHE = {}


def build(DX, DO, FF, BL, n_cores=N_CORES, reps=1):
    """Per-core kernel. DX: contraction dim (may include bias tile), DO: output dim."""
    KT = DX // 128
    NT = BL // 128
    FC = 512
    NFC = FF // FC
    NG = FF // GE
    GPC = FC // GE

    nc = bacc.Bacc("TRN2", target_bir_lowering=False, debug=False, num_devices=n_cores)
    xt = nc.dram_tensor("xt", [DX, BL], f16, kind="ExternalInput")
    xt8 = nc.dram_tensor("xt8", [2 * DX, BL], f8, kind="ExternalInput")
    wencT = nc.dram_tensor("wencT", [DX, FF], f16, kind="ExternalInput")
    wenc8 = nc.dram_tensor("wenc8", [DX, FF], f8, kind="ExternalInput")
    wdecT = nc.dram_tensor("wdecT", [FF, DO], f16, kind="ExternalInput")
    out = nc.dram_tensor("out", [BL, DO], f32, kind="ExternalOutput")

    wencT_r = wencT.ap().rearrange("(k p) f -> p k f", p=128)
    wenc8_r = wenc8.ap().rearrange("(k p) f -> p k f", p=128)
    xt_r = xt.ap().rearrange("(k p) b -> p k b", p=128)
    xt8_r = xt8.ap().rearrange("(k p) b -> p k b", p=128)

    with TileContext(nc) as tc:
        with (
            tc.tile_pool(name="dram", bufs=1, space="DRAM") as dpool,
            tc.tile_pool(name="xt_sb", bufs=1) as xpool,
            tc.tile_pool(name="const", bufs=1) as kpool,
            tc.tile_pool(name="wenc", bufs=2) as wpool,
            tc.tile_pool(name="apsum", bufs=2, space="PSUM") as apsum,
            tc.tile_pool(name="abounce", bufs=4) as apool,
            tc.tile_pool(name="gbuf", bufs=4) as gpool,
            tc.tile_pool(name="cand", bufs=1) as cpool,
            tc.tile_pool(name="pack", bufs=1) as ppool,
            tc.tile_pool(name="small", bufs=4) as spool,
            tc.tile_pool(name="wdecg", bufs=2) as wgpool,
            tc.tile_pool(name="dpsum", bufs=2, space="PSUM") as dpsum,
            tc.tile_pool(name="cout", bufs=1) as opool,
        ):
            preD = dpool.tile([BL, FF], f32)
            preD_g = preD[:, :].rearrange("b (g e) -> (b g) e", e=GE)

            xt_sb = xpool.tile([128, KT, BL], f16)
            nc.sync.dma_start(out=xt_sb[:], in_=xt_r)
            xt8_sb = xpool.tile([128, 2 * KT, BL], f8)
            nc.sync.dma_start(out=xt8_sb[:], in_=xt8_r)
            ident = kpool.tile([128, 128], f16)
            make_identity(nc, ident[:])
            gid = kpool.tile([128, NG], i32)
            nc.gpsimd.iota(gid[:], pattern=[[1, NG]], base=0, channel_multiplier=0)
            tagi = kpool.tile([128, NP, GE], i32)
            nc.gpsimd.iota(tagi[:], pattern=[[0, NP], [1, GE]], base=0,
                           channel_multiplier=0)

            def pair_phase_a(t0):
                # two row-tiles share each W chunk load (halves W traffic)
                Gs = [gpool.tile([128, NG], f16, name=f"G{_i}") for _i in range(2)]
                for fc in range(NFC):
                    w = wpool.tile([128, KT, FC], f16, name="w")
                    nc.sync.dma_start(out=w[:], in_=wencT_r[:, :, ds(fc * FC, FC)])
                    w8 = wpool.tile([128, 2 * KT, FC], f8, name="w8")
                    # first half (Wh*2^4) derived on-chip from the fp16 tile;
                    # only the Wl*2^19 half streams from DRAM
                    nc.scalar.activation(w8[:, 0:KT, :], w[:, :, :], Act.Copy,
                                         scale=16.0)
                    nc.sync.dma_start(out=w8[:, ds(KT, KT), :],
                                      in_=wenc8_r[:, :, ds(fc * FC, FC)])
                    for ti in range(2):
                        t = t0 + ti
                        rows = ts(t, 128)
                        ps = apsum.tile([128, FC], f32, name="ps")
                        for k in range(KT):
                            nc.tensor.matmul(
                                ps[:], lhsT=xt_sb[:, k, rows], rhs=w[:, k, :],
                                start=(k == 0), stop=(k == KT - 1),
                            )
                        # residual (xl*2^11 . Wh*2^4) + (xh*2^-4 . Wl*2^19), fp8 DoubleRow
                        ps2 = apsum.tile([128, FC], f32, name="ps2")
                        for k in range(0, 2 * KT, 2):
                            nc.tensor.matmul(
                                ps2[:], lhsT=xt8_sb[:, k:k + 2, rows],
                                rhs=w8[:, k:k + 2, :],
                                start=(k == 0), stop=(k == 2 * KT - 2),
                                perf_mode=mybir.MatmulPerfMode.DoubleRow,
                            )
                        r2 = apool.tile([128, FC], f32, name="r2")
                        nc.scalar.activation(r2[:], ps2[:], Act.Copy, scale=2.0 ** -15)
                        a = apool.tile([128, FC], f32, name="a")
                        nc.vector.tensor_tensor(out=a[:], in0=ps[:], in1=r2[:],
                                                op=Alu.add)
                        nc.sync.dma_start(out=preD[rows, ds(fc * FC, FC)], in_=a[:])
                        av = a[:, :].rearrange("p (g e) -> p g e", e=GE)
                        nc.vector.reduce_max(
                            out=Gs[ti][:, ds(fc * GPC, GPC)], in_=av,
                            axis=mybir.AxisListType.X)
                return Gs

            def tile_body(t, G):
                rows = ts(t, 128)
                # ---------- Phase B: exact top-64 ----------
                nc.vector.tensor_scalar(out=G[:], in0=G[:], scalar1=0.0,
                                        scalar2=None, op0=Alu.max)
                gpk = ppool.tile([128, NG], u32, name="gpk")
                nc.vector.tensor_copy(gpk[:], G[:, :].bitcast(u16))
                nc.vector.tensor_scalar(out=gpk[:], in0=gpk[:], scalar1=16,
                                        scalar2=None, op0=Alu.logical_shift_left)
                nc.vector.tensor_tensor(out=gpk[:], in0=gpk[:], in1=gid[:, :].bitcast(u32),
                                        op=Alu.bitwise_or)
                gpkf = gpk[:, :].bitcast(f32)
                gtop = spool.tile([128, NP], f32, name="gtop")
                for r in range(NP // 8):
                    mv = gtop[:, ds(r * 8, 8)]
                    nc.vector.max(out=mv, in_=gpkf)
                    if r < NP // 8 - 1:
                        nc.vector.match_replace(out=gpkf, in_to_replace=mv,
                                                in_values=gpkf, imm_value=0.0)
                gsel = spool.tile([128, NP], u32, name="gsel")
                nc.vector.tensor_scalar(out=gsel[:], in0=gtop[:, :].bitcast(u32),
                                        scalar1=0xFFFF, scalar2=None, op0=Alu.bitwise_and)
                goff = spool.tile([128, NP], i32, name="goff")
                nc.gpsimd.iota(goff[:], pattern=[[0, NP]], base=t * 128 * NG,
                               channel_multiplier=NG)
                nc.vector.tensor_tensor(out=goff[:], in0=goff[:], in1=gsel[:, :].bitcast(i32),
                                        op=Alu.add)
                cand = cpool.tile([128, NP, GE], f32, name="cand")
                for cb in range(NP):
                    nc.gpsimd.indirect_dma_start(
                        out=cand[:, cb, :], out_offset=None, in_=preD_g,
                        in_offset=bass.IndirectOffsetOnAxis(ap=goff[:, cb:cb + 1], axis=0),
                    )
                gsel_b = gsel[:, :].rearrange("p (n o) -> p n o", o=1).to_broadcast([128, NP, GE])
                tagm = ppool.tile([128, NP, GE], u32, name="tagm")
                nc.vector.tensor_scalar(out=tagm[:], in0=gsel_b, scalar1=GE,
                                        scalar2=None, op0=Alu.mult)
                nc.vector.tensor_tensor(out=tagm[:], in0=tagm[:], in1=tagi[:, :, :].bitcast(u32),
                                        op=Alu.add)
                cand2 = cand[:, :, :].rearrange("p n e -> p (n e)")
                cbf = cpool.tile([128, NP * GE], f16, name="cbf")
                nc.vector.tensor_copy(cbf[:], cand2)
                cpk = cpool.tile([128, NP * GE], u32, name="cpk")
                nc.vector.tensor_copy(cpk[:], cbf[:, :].bitcast(u16))
                nc.vector.tensor_scalar(out=cpk[:], in0=cpk[:], scalar1=16,
                                        scalar2=None, op0=Alu.logical_shift_left)
                tagm2 = tagm[:, :, :].rearrange("p n e -> p (n e)")
                nc.vector.tensor_tensor(out=cpk[:], in0=cpk[:], in1=tagm2, op=Alu.bitwise_or)
                # exact t* from f32 candidate values (destructive rounds on a copy)
                vr = cpool.tile([128, NP * GE], f32, name="vr")
                nc.vector.tensor_copy(vr[:], cand2)
                mvf = None
                for r in range(K // 8):
                    mvf = spool.tile([128, 8], f32, name="mvf")
                    nc.vector.max(out=mvf[:], in_=vr[:])
                    if r < K // 8 - 1:
                        nc.vector.match_replace(out=vr[:], in_to_replace=mvf[:],
                                                in_values=vr[:], imm_value=0.0)
                tstar = spool.tile([128, 1], f32, name="tstar")
                nc.vector.tensor_copy(tstar[:], mvf[:, 7:8])
                # mask packed array to the exact selection, then extract pairs
                cpkf = cpk[:, :].bitcast(f32)
                nc.vector.scalar_tensor_tensor(
                    out=cpkf, in0=cand2, scalar=tstar[:], in1=cpkf,
                    op0=Alu.is_ge, op1=Alu.mult,
                )
                pk = spool.tile([128, K], f32, name="pk")
                for r in range(K // 8):
                    mv = pk[:, ds(r * 8, 8)]
                    nc.vector.max(out=mv, in_=cpkf)
                    if r < K // 8 - 1:
                        nc.vector.match_replace(out=cpkf, in_to_replace=mv,
                                                in_values=cpkf, imm_value=0.0)
                fsel = spool.tile([128, K], u32, name="fsel")
                nc.vector.tensor_scalar(out=fsel[:], in0=pk[:, :].bitcast(u32),
                                        scalar1=0xFFFF, scalar2=None, op0=Alu.bitwise_and)
                wbits = spool.tile([128, K], u32, name="wbits")
                nc.vector.tensor_scalar(out=wbits[:], in0=pk[:, :].bitcast(u32),
                                        scalar1=16, scalar2=None,
                                        op0=Alu.logical_shift_right)
                wnarrow = spool.tile([128, K], u16, name="wnarrow")
                nc.vector.tensor_copy(wnarrow[:], wbits[:])
                wsel = spool.tile([128, K], f32, name="wsel")
                nc.vector.tensor_copy(wsel[:], wnarrow[:, :].bitcast(f16))

                # ---------- Phase C: gather W_dec rows, accumulate on PE ----
                ND2 = max(1, DO // 512)
                DW = DO // ND2
                psD = [dpsum.tile([128, DW], f32, name=f"psD{_h}") for _h in range(ND2)]
                for blk in range(K // PB):
                    wg = wgpool.tile([128, PB, DO], f16, name="wg")
                    for j2 in range(PB):
                        k2 = blk * PB + j2
                        nc.gpsimd.indirect_dma_start(
                            out=wg[:, j2, :], out_offset=None, in_=wdecT[:, :],
                            in_offset=bass.IndirectOffsetOnAxis(
                                ap=fsel[:, k2:k2 + 1], axis=0),
                        )
                    for j in range(PB):
                        k = blk * PB + j
                        dg = apool.tile([128, 128], f16, name="dg")
                        nc.vector.tensor_scalar(out=dg[:], in0=ident[:],
                                                scalar1=wsel[:, k:k + 1], scalar2=None,
                                                op0=Alu.mult)
                        first = (k == 0)
                        last = (k == K - 1)
                        for h in range(ND2):
                            nc.tensor.matmul(psD[h][:], lhsT=dg[:],
                                             rhs=wg[:, j, ds(h * DW, DW)],
                                             start=first, stop=last)
                co = opool.tile([128, DO], f32, name="co")
                for h in range(ND2):
                    nc.vector.tensor_copy(co[:, ds(h * DW, DW)], psD[h][:])
                nc.sync.dma_start(out=out.ap()[rows, :], in_=co[:])

            def full_pass():
                for t0 in range(0, NT, 2):
                    Gs = pair_phase_a(t0)
                    tile_body(t0, Gs[0])
                    tile_body(t0 + 1, Gs[1])

            if reps > 1:
                with tc.For_i(0, reps, 1):
                    full_pass()
            else:
                full_pass()

    nc.compile()
    return nc


def get_kernel(DX, reps=1):
    key = (DX, reps)
    if key not in _CACHE:
        _CACHE[key] = build(DX, D, F, B // N_CORES, N_CORES, reps=reps)
    return _CACHE[key]


def prep_in_maps(x, W_enc, b_enc, W_dec, b_dec):
    BL = B // N_CORES
    xs = (x - b_dec).astype(np.float32)
    wencT = np.ascontiguousarray(W_enc.T.astype(np.float32))   # [D, F]
    if np.any(b_enc):
        # fold b_enc in as one extra 128-row contraction tile
        DX = D + 128
        xa = np.zeros((B, DX), np.float32)
        xa[:, :D] = xs
        xa[:, D] = 1.0
        wa = np.zeros((DX, F), np.float32)
        wa[:D] = wencT
        wa[D] = b_enc
        xs, wencT = xa, wa
    else:
        DX = D
    import ml_dtypes
    f8np = ml_dtypes.float8_e4m3
    xst = np.ascontiguousarray(xs.T)                            # [DX, B]
    wdecT = np.ascontiguousarray(W_dec.T).astype(np.float16)    # [F, D]
    # fp16 main + fp8 residual operands:
    #   pre = xh.Wh + 2^-15 * ((xl*2^11).(Wh*2^4) + (xh*2^-4).(Wl*2^19))
    wh = wencT.astype(np.float16)
    wl = wencT - wh.astype(np.float32)
    wenc8 = (wl * 2.0 ** 19).astype(f8np)                       # [DX, F]
    xh = xst.astype(np.float16)
    xl = xst - xh.astype(np.float32)
    xt8 = np.concatenate([
        (xl * 2.0 ** 11).astype(f8np),
        (xh.astype(np.float32) * 2.0 ** -4).astype(f8np)])      # [2*DX, B]
    in_maps = [{
        "xt": np.ascontiguousarray(xh[:, c * BL:(c + 1) * BL]),
        "xt8": np.ascontiguousarray(xt8[:, c * BL:(c + 1) * BL]),
        "wencT": wh,
        "wenc8": wenc8,
        "wdecT": wdecT,
    } for c in range(N_CORES)]
    return in_maps, DX


def kernel(x, W_enc, b_enc, W_dec, b_dec):
    x = np.asarray(x, np.float32)
    W_enc = np.asarray(W_enc, np.float32)
    b_enc = np.asarray(b_enc, np.float32)
    W_dec = np.asarray(W_dec, np.float32)
    b_dec = np.asarray(b_dec, np.float32)
    in_maps, DX = prep_in_maps(x, W_enc, b_enc, W_dec, b_dec)
    nc = get_kernel(DX)
    res = run_bass_kernel_spmd(nc, in_maps, list(range(N_CORES)))
    y = np.concatenate([res.results[c]["out"] for c in range(N_CORES)], axis=0)
    return (y + b_dec).astype(np.float32)


# revision 18
# speedup vs baseline: 1.0170x; 1.0170x over previous
"""TopK autoencoder (B=4096, D=1024, F=32768, K=64) on 8 Trainium2 NeuronCores.

Strategy: data-parallel over batch (512 rows/core). Per core, per 128-row tile:
  A) encoder matmul (PE): fp16 main pass (xh.Wh) + fp8e4m3 DoubleRow residual
     pass computing 2^15*(xl.Wh + xh.Wl) via pre-scaled operands; combined on
     ACT/DVE (pre-act abs err ~7e-6) and spilled raw to DRAM in f32; fused
     per-group (32 elems) running max on DVE.
  B) top-K: group maxima clamped at 0 and packed as
     (fp16-value-bits << 16 | group-id) so max8/match_replace rounds are
     tie-free; top-80 groups gathered from the spilled pre-activations by
     per-column indirect DMAs (multi-offset indirect DMA crashes the HW
     runtime); candidates packed the same way with element tags; K-th
     largest of the f32 candidates = threshold; masked packed rounds
     extract the top-64 (value, index) pairs.
  C) decode: gather the selected W_dec rows (fp16) by index per-column and
     accumulate w_k * row_k on the PE via diagonal-matrix matmuls into PSUM.
b_dec is handled exactly on the host (x - b_dec, + b_dec at the end); a
nonzero b_enc is folded in as an extra contraction tile (zero here).
"""
import sys
sys.path.insert(0, '/opt/trn_rl_repo')
import numpy as np
import concourse.bass as bass
import concourse.mybir as mybir
from concourse import bacc
from concourse.bass import ts, ds
from concourse.tile import TileContext
from concourse.masks import make_identity
from concourse.bass_utils import run_bass_kernel_spmd

f32 = mybir.dt.float32
f16 = mybir.dt.float16
f8 = mybir.dt.float8e4
u16 = mybir.dt.uint16
u32 = mybir.dt.uint32
i32 = mybir.dt.int32
Alu = mybir.AluOpType
Act = mybir.ActivationFunctionType

B, D, F, K = 4096, 1024, 32768, 64
N_CORES = 8
GE = 32     # group size
NP = 80     # candidate groups per row (64 needed + tie margin)
PB = 8      # decode gather block (rows per indirect DMA)

_CACHE = {}


def build(DX, DO, FF, BL, n_cores=N_CORES, reps=1):
    """Per-core kernel. DX: contraction dim (may include bias tile), DO: output dim."""
    KT = DX // 128
    NT = BL // 128
    FC = 512
    NFC = FF // FC
    NG = FF // GE
    GPC = FC // GE

    nc = bacc.Bacc("TRN2", target_bir_lowering=False, debug=False, num_devices=n_cores)
    xt = nc.dram_tensor("xt", [DX, BL], f16, kind="ExternalInput")
    xt8 = nc.dram_tensor("xt8", [2 * DX, BL], f8, kind="ExternalInput")
    wencT = nc.dram_tensor("wencT", [DX, FF], f16, kind="ExternalInput")
    wenc8 = nc.dram_tensor("wenc8", [2 * DX, FF], f8, kind="ExternalInput")
    wdecT = nc.dram_tensor("wdecT", [FF, DO], f16, kind="ExternalInput")
    out = nc.dram_tensor("out", [BL, DO], f32, kind="ExternalOutput")

    wencT_r = wencT.ap().rearrange("(k p) f -> p k f", p=128)
    wenc8_r = wenc8.ap().rearrange("(k p) f -> p k f", p=128)
    xt_r = xt.ap().rearrange("(k p) b -> p k b", p=128)
    xt8_r = xt8.ap().rearrange("(k p) b -> p k b", p=128)

    with TileContext(nc) as tc:
        with (
            tc.tile_pool(name="dram", bufs=1, space="DRAM") as dpool,
            tc.tile_pool(name="xt_sb", bufs=1) as xpool,
            tc.tile_pool(name="const", bufs=1) as kpool,
            tc.tile_pool(name="wenc", bufs=3) as wpool,
            tc.tile_pool(name="apsum", bufs=2, space="PSUM") as apsum,
            tc.tile_pool(name="abounce", bufs=5) as apool,
            tc.tile_pool(name="gbuf", bufs=4) as gpool,
            tc.tile_pool(name="cand", bufs=1) as cpool,
            tc.tile_pool(name="pack", bufs=1) as ppool,
            tc.tile_pool(name="small", bufs=3) as spool,
            tc.tile_pool(name="wdecg", bufs=2) as wgpool,
            tc.tile_pool(name="dpsum", bufs=2, space="PSUM") as dpsum,
            tc.tile_pool(name="cout", bufs=1) as opool,
        ):
            preD = dpool.tile([BL, FF], f32)
            preD_g = preD[:, :].rearrange("b (g e) -> (b g) e", e=GE)

            xt_sb = xpool.tile([128, KT, BL], f16)
            nc.sync.dma_start(out=xt_sb[:], in_=xt_r)
            xt8_sb = xpool.tile([128, 2 * KT, BL], f8)
            nc.sync.dma_start(out=xt8_sb[:], in_=xt8_r)
            ident = kpool.tile([128, 128], f16)
            make_identity(nc, ident[:])
            gid = kpool.tile([128, NG], i32)
            nc.gpsimd.iota(gid[:], pattern=[[1, NG]], base=0, channel_multiplier=0)
            tagi = kpool.tile([128, NP, GE], i32)
            nc.gpsimd.iota(tagi[:], pattern=[[0, NP], [1, GE]], base=0,
                           channel_multiplier=0)

            def pair_phase_a(t0):
                # two row-tiles share each W chunk load (halves W traffic)
                Gs = [gpool.tile([128, NG], f16, name=f"G{_i}") for _i in range(2)]
                for fc in range(NFC):
                    w = wpool.tile([128, KT, FC], f16, name="w")
                    nc.sync.dma_start(out=w[:], in_=wencT_r[:, :, ds(fc * FC, FC)])
                    w8 = wpool.tile([128, 2 * KT, FC], f8, name="w8")
                    nc.sync.dma_start(out=w8[:], in_=wenc8_r[:, :, ds(fc * FC, FC)])
                    for ti in range(2):
                        t = t0 + ti
                        rows = ts(t, 128)
                        ps = apsum.tile([128, FC], f32, name="ps")
                        for k in range(KT):
                            nc.tensor.matmul(
                                ps[:], lhsT=xt_sb[:, k, rows], rhs=w[:, k, :],
                                start=(k == 0), stop=(k == KT - 1),
                            )
                        # residual (xl*2^11 . Wh*2^4) + (xh*2^-4 . Wl*2^19), fp8 DoubleRow
                        ps2 = apsum.tile([128, FC], f32, name="ps2")
                        for k in range(0, 2 * KT, 2):
                            nc.tensor.matmul(
                                ps2[:], lhsT=xt8_sb[:, k:k + 2, rows],
                                rhs=w8[:, k:k + 2, :],
                                start=(k == 0), stop=(k == 2 * KT - 2),
                                perf_mode=mybir.MatmulPerfMode.DoubleRow,
                            )
                        r2 = apool.tile([128, FC], f32, name="r2")
                        nc.scalar.activation(r2[:], ps2[:], Act.Copy, scale=2.0 ** -15)
                        a = apool.tile([128, FC], f32, name="a")
                        nc.vector.tensor_tensor(out=a[:], in0=ps[:], in1=r2[:],
                                                op=Alu.add)
                        nc.sync.dma_start(out=preD[rows, ds(fc * FC, FC)], in_=a[:])
                        av = a[:, :].rearrange("p (g e) -> p g e", e=GE)
                        nc.vector.reduce_max(
                            out=Gs[ti][:, ds(fc * GPC, GPC)], in_=av,
                            axis=mybir.AxisListType.X)
                return Gs

            def tile_body(t, G):
                rows = ts(t, 128)
                # ---------- Phase B: exact top-64 ----------
                nc.vector.tensor_scalar(out=G[:], in0=G[:], scalar1=0.0,
                                        scalar2=None, op0=Alu.max)
                gpk = ppool.tile([128, NG], u32, name="gpk")
                nc.vector.tensor_copy(gpk[:], G[:, :].bitcast(u16))
                nc.vector.tensor_scalar(out=gpk[:], in0=gpk[:], scalar1=16,
                                        scalar2=None, op0=Alu.logical_shift_left)
                nc.vector.tensor_tensor(out=gpk[:], in0=gpk[:], in1=gid[:, :].bitcast(u32),
                                        op=Alu.bitwise_or)
                gpkf = gpk[:, :].bitcast(f32)
                gtop = spool.tile([128, NP], f32, name="gtop")
                for r in range(NP // 8):
                    mv = gtop[:, ds(r * 8, 8)]
                    nc.vector.max(out=mv, in_=gpkf)
                    if r < NP // 8 - 1:
                        nc.vector.match_replace(out=gpkf, in_to_replace=mv,
                                                in_values=gpkf, imm_value=0.0)
                gsel = spool.tile([128, NP], u32, name="gsel")
                nc.vector.tensor_scalar(out=gsel[:], in0=gtop[:, :].bitcast(u32),
                                        scalar1=0xFFFF, scalar2=None, op0=Alu.bitwise_and)
                goff = spool.tile([128, NP], i32, name="goff")
                nc.gpsimd.iota(goff[:], pattern=[[0, NP]], base=t * 128 * NG,
                               channel_multiplier=NG)
                nc.vector.tensor_tensor(out=goff[:], in0=goff[:], in1=gsel[:, :].bitcast(i32),
                                        op=Alu.add)
                cand = cpool.tile([128, NP, GE], f32, name="cand")
                for cb in range(NP):
                    nc.gpsimd.indirect_dma_start(
                        out=cand[:, cb, :], out_offset=None, in_=preD_g,
                        in_offset=bass.IndirectOffsetOnAxis(ap=goff[:, cb:cb + 1], axis=0),
                    )
                gsel_b = gsel[:, :].rearrange("p (n o) -> p n o", o=1).to_broadcast([128, NP, GE])
                tagm = ppool.tile([128, NP, GE], u32, name="tagm")
                nc.vector.tensor_scalar(out=tagm[:], in0=gsel_b, scalar1=GE,
                                        scalar2=None, op0=Alu.mult)
                nc.vector.tensor_tensor(out=tagm[:], in0=tagm[:], in1=tagi[:, :, :].bitcast(u32),
                                        op=Alu.add)
                cand2 = cand[:, :, :].rearrange("p n e -> p (n e)")
                cbf = cpool.tile([128, NP * GE], f16, name="cbf")
                nc.vector.tensor_copy(cbf[:], cand2)
                cpk = cpool.tile([128, NP * GE], u32, name="cpk")
                nc.vector.tensor_copy(cpk[:], cbf[:, :].bitcast(u16))
                nc.vector.tensor_scalar(out=cpk[:], in0=cpk[:], scalar1=16,
                                        scalar2=None, op0=Alu.logical_shift_left)
                tagm2 = tagm[:, :, :].rearrange("p n e -> p (n e)")
                nc.vector.tensor_tensor(out=cpk[:], in0=cpk[:], in1=tagm2, op=Alu.bitwise_or)
                # exact t* from f32 candidate values (destructive rounds on a copy)
                vr = cpool.tile([128, NP * GE], f32, name="vr")
                nc.vector.tensor_copy(vr[:], cand2)
                mvf = None
                for r in range(K // 8):
                    mvf = spool.tile([128, 8], f32, name="mvf")
                    nc.vector.max(out=mvf[:], in_=vr[:])
                    if r < K // 8 - 1:
                        nc.vector.match_replace(out=vr[:], in_to_replace=mvf[:],
                                                in_values=vr[:], imm_value=0.0)
                tstar = spool.tile([128, 1], f32, name="tstar")
                nc.vector.tensor_copy(tstar[:], mvf[:, 7:8])
                # mask packed array to the exact selection, then extract pairs
                cpkf = cpk[:, :].bitcast(f32)
                nc.vector.scalar_tensor_tensor(
                    out=cpkf, in0=cand2, scalar=tstar[:], in1=cpkf,
                    op0=Alu.is_ge, op1=Alu.mult,
                )
                pk = spool.tile([128, K], f32, name="pk")
                for r in range(K // 8):
                    mv = pk[:, ds(r * 8, 8)]
                    nc.vector.max(out=mv, in_=cpkf)
                    if r < K // 8 - 1:
                        nc.vector.match_replace(out=cpkf, in_to_replace=mv,
                                                in_values=cpkf, imm_value=0.0)
                fsel = spool.tile([128, K], u32, name="fsel")
                nc.vector.tensor_scalar(out=fsel[:], in0=pk[:, :].bitcast(u32),
                                        scalar1=0xFFFF, scalar2=None, op0=Alu.bitwise_and)
                wbits = spool.tile([128, K], u32, name="wbits")
                nc.vector.tensor_scalar(out=wbits[:], in0=pk[:, :].bitcast(u32),
                                        scalar1=16, scalar2=None,
                                        op0=Alu.logical_shift_right)
                wnarrow = spool.tile([128, K], u16, name="wnarrow")
                nc.vector.tensor_copy(wnarrow[:], wbits[:])
                wsel = spool.tile([128, K], f32, name="wsel")
                nc.vector.tensor_copy(wsel[:], wnarrow[:, :].bitcast(f16))

                # ---------- Phase C: gather W_dec rows, accumulate on PE ----
                ND2 = max(1, DO // 512)
                DW = DO // ND2
                psD = [dpsum.tile([128, DW], f32, name=f"psD{_h}") for _h in range(ND2)]
                for blk in range(K // PB):
                    wg = wgpool.tile([128, PB, DO], f16, name="wg")
                    for j2 in range(PB):
                        k2 = blk * PB + j2
                        nc.gpsimd.indirect_dma_start(
                            out=wg[:, j2, :], out_offset=None, in_=wdecT[:, :],
                            in_offset=bass.IndirectOffsetOnAxis(
                                ap=fsel[:, k2:k2 + 1], axis=0),
                        )
                    for j in range(PB):
                        k = blk * PB + j
                        dg = apool.tile([128, 128], f16, name="dg")
                        nc.vector.tensor_scalar(out=dg[:], in0=ident[:],
                                                scalar1=wsel[:, k:k + 1], scalar2=None,
                                                op0=Alu.mult)
                        first = (k == 0)
                        last = (k == K - 1)
                        for h in range(ND2):
                            nc.tensor.matmul(psD[h][:], lhsT=dg[:],
                                             rhs=wg[:, j, ds(h * DW, DW)],
                                             start=first, stop=last)
                co = opool.tile([128, DO], f32, name="co")
                for h in range(ND2):
                    nc.vector.tensor_copy(co[:, ds(h * DW, DW)], psD[h][:])
                nc.sync.dma_start(out=out.ap()[rows, :], in_=co[:])

            def full_pass():
                for t0 in range(0, NT, 2):
                    Gs = pair_phase_a(t0)
                    tile_body(t0, Gs[0])
                    tile_body(t0 + 1, Gs[1])

            if reps > 1:
                with tc.For_i(0, reps, 1):
                    full_pass()
            else:
                full_pass()

    nc.compile()
    return nc


def get_kernel(DX, reps=1):
    key = (DX, reps)
    if key not in _CACHE:
        _CACHE[key] = build(DX, D, F, B // N_CORES, N_CORES, reps=reps)
    return _CACHE[key]


def prep_in_maps(x, W_enc, b_enc, W_dec, b_dec):
    BL = B // N_CORES
    xs = (x - b_dec).astype(np.float32)
    wencT = np.ascontiguousarray(W_enc.T.astype(np.float32))   # [D, F]
    if np.any(b_enc):
        # fold b_enc in as one extra 128-row contraction tile
        DX = D + 128
        xa = np.zeros((B, DX), np.float32)
        xa[:, :D] = xs
        xa[:, D] = 1.0
        wa = np.zeros((DX, F), np.float32)
        wa[:D] = wencT
        wa[D] = b_enc
        xs, wencT = xa, wa
    else:
        DX = D
    import ml_dtypes
    f8np = ml_dtypes.float8_e4m3
    xst = np.ascontiguousarray(xs.T)                            # [DX, B]
    wdecT = np.ascontiguousarray(W_dec.T).astype(np.float16)    # [F, D]
    # fp16 main + fp8 residual operands:
    #   pre = xh.Wh + 2^-15 * ((xl*2^11).(Wh*2^4) + (xh*2^-4).(Wl*2^19))
    wh = wencT.astype(np.float16)
    wl = wencT - wh.astype(np.float32)
    wenc8 = np.concatenate([
        (wh.astype(np.float32) * 2.0 ** 4).astype(f8np),
        (wl * 2.0 ** 19).astype(f8np)])                         # [2*DX, F]
    xh = xst.astype(np.float16)
    xl = xst - xh.astype(np.float32)
    xt8 = np.concatenate([
        (xl * 2.0 ** 11).astype(f8np),
        (xh.astype(np.float32) * 2.0 ** -4).astype(f8np)])      # [2*DX, B]
    in_maps = [{
        "xt": np.ascontiguousarray(xh[:, c * BL:(c + 1) * BL]),
        "xt8": np.ascontiguousarray(xt8[:, c * BL:(c + 1) * BL]),
        "wencT": wh,
        "wenc8": wenc8,
        "wdecT": wdecT,
    } for c in range(N_CORES)]
    return in_maps, DX


def kernel(x, W_enc, b_enc, W_dec, b_dec):
    x = np.asarray(x, np.float32)
    W_enc = np.asarray(W_enc, np.float32)
    b_enc = np.asarray(b_enc, np.float32)
    W_dec = np.asarray(W_dec, np.float32)
    b_dec = np.asarray(b_dec, np.float32)
    in_maps, DX = prep_in_maps(x, W_enc, b_enc, W_dec, b_dec)
    nc = get_kernel(DX)
    res = run_bass_kernel_spmd(nc, in_maps, list(range(N_CORES)))
    y = np.concatenate([res.results[c]["out"] for c in range(N_CORES)], axis=0)
    return (y + b_dec).astype(np.float32)
